# revision 28
# baseline (speedup 1.0000x reference)
"""Trainium2 Bass kernel for nn_BranchGCN (gnn_message_passing).

Two SPMD launches over 8 cores:
  Stage A -- node-model-parallel: core c owns nodes [4c, 4c+4) and streams its
    1/8 slice of W_branch (fp16) while computing per point: x, xx2=|x|^2/2,
    y = M1^T x, u = (M2-M1)^T x + bias + zc, plus an error-compensated fp16
    split (xh + xl) of x and xx2 for stage B's distance matmul. Weight-only
    folds (Wl1@Wl2, conv factorization M1/M2/zc) are host precomputed; all
    per-sample math stays on device.
  Host    -- pure relayout (numpy transposes / dtype casts / constant fills).
  Stage B -- row-sharded KNN EdgeConv. Per 128-row tile, one K=12 fp16 matmul
    (split-compensated, fp32-grade accuracy) yields
      pd[n,j] = x_n.x_j - |x_j|^2/2 - |x_n|^2/2  (~ -|x_n-x_j|^2/2, row-shift)
    Exact top-8 via two pairwise-max folds (2048->512 slots), max8+max_index
    on the folded array, then 4-way slot expansion: the true top-8 provably
    lies among the 32 expanded candidates. One multi-offset indirect DMA
    gathers their [x, xx2, y] rows, pd is recomputed in fp32 for the 32,
    thresholded at the 8th largest, and max_k y + center term u -> leaky.
"""

import sys
import numpy as np

sys.path.insert(0, "/opt/trn_rl_repo")

from contextlib import ExitStack

import concourse.tile as tile
from concourse import bacc, bass, mybir
from concourse.bass_utils import run_bass_kernel_spmd

FP = mybir.dt.float32
FP16 = mybir.dt.float16
U32 = mybir.dt.uint32
ALU = mybir.AluOpType
AF = mybir.ActivationFunctionType

B, NODE, DEG, K = 16, 32, 64, 8
IN_F, OUT_F, SUP = 128, 3, 10
FEATS = [96, 256, 256, 256, 128, 128]
SIZES = [1, 2, 4, 8, 16, 32]
NCORES = 8
NLOC = NODE // NCORES          # 4 nodes per core
N = NODE * DEG                 # 2048 graph rows
RLOC = NLOC * DEG              # 256 rows per core
NP_CORE = B * RLOC             # 4096 points per core
NF = 512                       # fold slots (2048 / 4)
BIG = 60000.0
NCH = [(f + 127) // 128 for f in FEATS]


# --------------------------------------------------------------------------
# Stage A
# --------------------------------------------------------------------------
def build_stage_a():
    nc = bacc.Bacc(None)
    tlT = [nc.declare_dram_parameter(f"tlT{i}", [128, NCH[i] * NLOC * B], FP,
                                     isOutput=False) for i in range(6)]
    wrs = [nc.declare_dram_parameter(f"wr{i}", [128, NCH[i] * OUT_F], FP,
                                     isOutput=False) for i in range(6)]
    wb = nc.declare_dram_parameter("wb", [NLOC, IN_F, DEG * IN_F], FP16,
                                   isOutput=False)
    t5h = nc.declare_dram_parameter("t5h", [IN_F, NLOC, B], FP16,
                                    isOutput=False)
    wbig = nc.declare_dram_parameter("wbig", [IN_F, 16], FP16, isOutput=False)
    abig = nc.declare_dram_parameter("abig", [OUT_F, 16], FP16, isOutput=False)
    bsel = nc.declare_dram_parameter("bsel", [OUT_F, 16], FP16, isOutput=False)
    biasu = nc.declare_dram_parameter("biasu", [OUT_F, DEG], FP16,
                                      isOutput=False)
    # fp32 rows: 0-2 x, 3 xx2, 4-6 y, 8-10 u; cols (nl, b, d)
    xout = nc.declare_dram_parameter("xout", [16, NP_CORE], FP, isOutput=True)
    # fp16 rows: 0-2 xh, 3 xxh2, 4-6 xl, 7 xxl2
    xout16 = nc.declare_dram_parameter("xout16", [8, NP_CORE], FP16,
                                       isOutput=True)

    with tile.TileContext(nc) as tc, ExitStack() as ctx:
        sbp = ctx.enter_context(tc.tile_pool(name="sbuf", bufs=1))
        wbpool = ctx.enter_context(tc.tile_pool(name="wbuf", bufs=2))
        psp = ctx.enter_context(tc.tile_pool(name="psum", bufs=2,
                                             space="PSUM"))
        pbp = ctx.enter_context(tc.tile_pool(name="psumb", bufs=2,
                                             space="PSUM"))
        pxp = ctx.enter_context(tc.tile_pool(name="psumx", bufs=2,
                                             space="PSUM"))

        # ---- small loads
        tl_sb, wr_sb = [], []
        for i in range(6):
            t = sbp.tile([128, NCH[i], NLOC * B], FP, tag=f"tlT{i}")
            nc.sync.dma_start(out=t[:], in_=tlT[i][:].rearrange(
                "p (c n) -> p c n", c=NCH[i]))
            tl_sb.append(t)
            w = sbp.tile([128, NCH[i], OUT_F], FP, tag=f"wr{i}")
            nc.sync.dma_start(out=w[:], in_=wrs[i][:].rearrange(
                "p (c o) -> p c o", c=NCH[i]))
            wr_sb.append(w)
        t5_sb = sbp.tile([IN_F, NLOC, B], FP16)
        nc.sync.dma_start(out=t5_sb[:], in_=t5h[:])
        wbig_sb = sbp.tile([IN_F, 16], FP16)
        nc.sync.dma_start(out=wbig_sb[:], in_=wbig[:])
        abig_sb = sbp.tile([OUT_F, 16], FP16)
        nc.sync.dma_start(out=abig_sb[:], in_=abig[:])
        bsel_sb = sbp.tile([OUT_F, 16], FP16)
        nc.sync.dma_start(out=bsel_sb[:], in_=bsel[:])
        biasu_sb = sbp.tile([OUT_F, DEG], FP16)
        nc.sync.dma_start(out=biasu_sb[:], in_=biasu[:])
        oneh = sbp.tile([OUT_F, 1], FP)
        nc.vector.memset(oneh[:], 0.5)

        # ---- root aggregation: rootT fp16 (3, nl, b)
        proot = psp.tile([OUT_F, NLOC * B], FP, tag="proot")
        steps = [(i, c) for i in range(6) for c in range(NCH[i])]
        for si, (i, c) in enumerate(steps):
            nc.tensor.matmul(out=proot[:],
                             lhsT=wr_sb[i][:, c, :],
                             rhs=tl_sb[i][:, c, :],
                             start=(si == 0), stop=(si == len(steps) - 1))
        rootT = sbp.tile([OUT_F, NLOC, B], FP16)
        nc.scalar.activation(out=rootT[:],
                             in_=proot[:].rearrange("p (nl b) -> p nl b",
                                                    nl=NLOC),
                             func=AF.Copy)

        # ---- per-node: branch einsum (fp16) + leaky -> branchT fp16
        branchT = sbp.tile([IN_F, NLOC, B, 2, 32], FP16)
        xout_sb = sbp.tile([16, NLOC, B, DEG], FP)
        xx2_sb = sbp.tile([1, NLOC, B, DEG], FP)
        xh3_sb = sbp.tile([3, NLOC, B, DEG], FP16)
        xxh2_sb = sbp.tile([1, NLOC, B, DEG], FP16)
        xl3_sb = sbp.tile([3, NLOC, B, DEG], FP16)
        xxl2_sb = sbp.tile([1, NLOC, B, DEG], FP16)
        for nl in range(NLOC):
            wbt = wbpool.tile([IN_F, DEG * IN_F], FP16, tag="wbt")
            nc.sync.dma_start(out=wbt[:], in_=wb[nl])
            for g in range(2):
                pb = pbp.tile([IN_F, 32, B], FP, tag="pbranch")
                for dl in range(32):
                    d = g * 32 + dl
                    nc.tensor.matmul(out=pb[:, dl, :],
                                     lhsT=wbt[:, d * 128:(d + 1) * 128],
                                     rhs=t5_sb[:, nl, :],
                                     start=True, stop=True)
                # transpose-copy PSUM -> SBUF fp16 (b-major), then leaky
                cg = sbp.tile([IN_F, B, 32], FP16, tag="cg")
                nc.vector.tensor_copy(
                    out=cg[:].rearrange("p b dl -> p dl b"), in_=pb[:])
                nc.vector.scalar_tensor_tensor(
                    out=branchT[:, nl, :, g, :], in0=cg[:], scalar=0.2,
                    in1=cg[:], op0=ALU.mult, op1=ALU.max)
            # rows [x, y, u] for this node's points, 2 chunks of (8b x 64d)
            for h in range(2):
                pxo = pxp.tile([16, 8, DEG], FP, tag="pxo")
                nc.tensor.matmul(
                    out=pxo[:],
                    lhsT=wbig_sb[:],
                    rhs=branchT[:, nl, 8 * h:8 * h + 8, :, :].rearrange(
                        "p b g dl -> p (b g dl)"),
                    start=True, stop=False)
                nc.tensor.matmul(
                    out=pxo[:],
                    lhsT=abig_sb[:],
                    rhs=rootT[:, nl, 8 * h:8 * h + 8].unsqueeze(2)
                        .to_broadcast([OUT_F, 8, DEG]),
                    start=False, stop=False)
                nc.tensor.matmul(
                    out=pxo[:],
                    lhsT=bsel_sb[:],
                    rhs=biasu_sb[:].unsqueeze(1)
                        .to_broadcast([OUT_F, 8, DEG]),
                    start=False, stop=True)
                xo = xout_sb[:, nl, 8 * h:8 * h + 8, :]
                nc.scalar.activation(out=xo, in_=pxo[:], func=AF.Copy)
                # xx2 = 0.5 * sum x_c^2
                sq = sbp.tile([OUT_F, 8 * DEG], FP, tag="sq")
                nc.vector.scalar_tensor_tensor(
                    out=sq[:],
                    in0=xo[0:3].rearrange("p b d -> p (b d)"), scalar=0.0,
                    in1=xo[0:3].rearrange("p b d -> p (b d)"),
                    op0=ALU.bypass, op1=ALU.mult)
                pxx = psp.tile([1, 8 * DEG], FP, tag="pxx")
                nc.tensor.matmul(out=pxx[:], lhsT=oneh[:], rhs=sq[:],
                                 start=True, stop=True)
                nc.vector.tensor_copy(
                    out=xx2_sb[:, nl, 8 * h:8 * h + 8, :].rearrange(
                        "p b d -> p (b d)"), in_=pxx[:])
            # fp16 split of x and xx2 for this node (partition-0 aligned ops)
            nc.scalar.activation(out=xh3_sb[:, nl], in_=xout_sb[0:3, nl],
                                 func=AF.Copy)
            nc.scalar.activation(out=xxh2_sb[:, nl], in_=xx2_sb[:, nl],
                                 func=AF.Copy)
            nc.vector.scalar_tensor_tensor(
                out=xl3_sb[:, nl], in0=xout_sb[0:3, nl], scalar=0.0,
                in1=xh3_sb[:, nl], op0=ALU.bypass, op1=ALU.subtract)
            nc.vector.scalar_tensor_tensor(
                out=xxl2_sb[:, nl], in0=xx2_sb[:, nl], scalar=0.0,
                in1=xxh2_sb[:, nl], op0=ALU.bypass, op1=ALU.subtract)
            # per-node output stores (overlap with next node's wb load)
            CW = B * DEG
            nc.sync.dma_start(
                out=xout[0:3, nl * CW:(nl + 1) * CW],
                in_=xout_sb[0:3, nl].rearrange("p b d -> p (b d)"))
            nc.sync.dma_start(
                out=xout[4:11, nl * CW:(nl + 1) * CW],
                in_=xout_sb[4:11, nl].rearrange("p b d -> p (b d)"))
            nc.sync.dma_start(
                out=xout[3:4, nl * CW:(nl + 1) * CW],
                in_=xx2_sb[:, nl].rearrange("p b d -> p (b d)"))
            nc.sync.dma_start(
                out=xout16[0:3, nl * CW:(nl + 1) * CW],
                in_=xh3_sb[:, nl].rearrange("p b d -> p (b d)"))
            nc.sync.dma_start(
                out=xout16[3:4, nl * CW:(nl + 1) * CW],
                in_=xxh2_sb[:, nl].rearrange("p b d -> p (b d)"))
            nc.sync.dma_start(
                out=xout16[4:7, nl * CW:(nl + 1) * CW],
                in_=xl3_sb[:, nl].rearrange("p b d -> p (b d)"))
            nc.sync.dma_start(
                out=xout16[7:8, nl * CW:(nl + 1) * CW],
                in_=xxl2_sb[:, nl].rearrange("p b d -> p (b d)"))


    return nc


# --------------------------------------------------------------------------
# Stage B
# --------------------------------------------------------------------------
DEBUG_B = False
NF3 = 256   # level-3 fold slots


def build_stage_b():
    nc = bacc.Bacc(None)
    # k rows: 0-2 xh_j, 3-5 xl_j, 6-8 xh_j, 9 xxh2_j, 10 xxl2_j, 11 = -1
    vall = nc.declare_dram_parameter("vall", [12, B * N], FP16,
                                     isOutput=False)
    # k rows: 0-2 xh_n, 3-5 xh_n, 6-8 xl_n, 9-10 = -1, 11 xxh2_n
    uvl = nc.declare_dram_parameter("uvl", [12, NP_CORE], FP16,
                                    isOutput=False)
    rowdat = nc.declare_dram_parameter("rowdat", [128, 32 * 8], FP,
                                       isOutput=False)
    # ptab8[b][s, (u*4+q)*8 + c] = point j = s + 256u + 512q, vals
    # [x0, x1, x2, xx2, y0, y1, y2, 0]
    ptabs = [nc.declare_dram_parameter(f"ptab{b}", [NF3, 64], FP,
                                       isOutput=False) for b in range(B)]
    outc = nc.declare_dram_parameter("outc", [B, RLOC, OUT_F], FP,
                                     isOutput=True)
    if DEBUG_B:
        dbg_m3 = nc.declare_dram_parameter("dbg_m3", [128, NF3], FP16,
                                           isOutput=True)
        dbg_idx = nc.declare_dram_parameter("dbg_idx", [128, K], mybir.dt.uint16,
                                            isOutput=True)
        dbg_gth = nc.declare_dram_parameter("dbg_gth", [128, 8 * 64], FP,
                                            isOutput=True)
        dbg_pdc = nc.declare_dram_parameter("dbg_pdc", [128, K * 8], FP,
                                            isOutput=True)
        dbg_fr = nc.declare_dram_parameter("dbg_fr", [128, 32 * OUT_F], FP,
                                           isOutput=True)

    from concourse import library_config
    U16 = mybir.dt.uint16
    I16 = mybir.dt.int16

    with tile.TileContext(nc) as tc, ExitStack() as ctx:
        sbp = ctx.enter_context(tc.tile_pool(name="sbuf", bufs=1))
        lop = ctx.enter_context(tc.tile_pool(name="loop", bufs=4))
        bp = ctx.enter_context(tc.tile_pool(name="bloop", bufs=3))
        dramp = ctx.enter_context(tc.tile_pool(name="dram", bufs=2,
                                               space="DRAM"))
        pspd = ctx.enter_context(tc.tile_pool(name="pspd", bufs=2,
                                              space="PSUM"))

        nc.gpsimd.load_library(library_config.mlp)
        vall_sb = sbp.tile([12, B, N], FP16)
        nc.sync.dma_start(out=vall_sb[:],
                          in_=vall[:].rearrange("p (b n) -> p b n", b=B))
        uvl_sb = sbp.tile([12, NP_CORE], FP16)
        nc.sync.dma_start(out=uvl_sb[:], in_=uvl[:])
        rd_sb = sbp.tile([128, 32, 8], FP)
        nc.sync.dma_start(out=rd_sb[:],
                          in_=rowdat[:].rearrange("p (t v) -> p t v", t=32))
        final_raw = sbp.tile([128, 32, OUT_F], FP)

        for b in range(B):
            idx2 = bp.tile([128, 2, K], U16, tag="idx2")
            for m in range(2):
                t = b * 2 + m
                # ---- pd matmul (K=12 fp16 split-compensated)
                ppd = pspd.tile([128, N], FP, tag="ppd")
                for ch in range(4):
                    nc.tensor.matmul(
                        out=ppd[:, ch * NF:(ch + 1) * NF],
                        lhsT=uvl_sb[:, t * 128:(t + 1) * 128],
                        rhs=vall_sb[:, b, ch * NF:(ch + 1) * NF],
                        start=True, stop=True)
                # ---- folds 2048 -> 256:
                # m3[s] = max_{u,q} pd[s + 256u + 512q]
                call = lop.tile([128, 3, NF], FP16, tag="call")
                nc.scalar.activation(
                    out=call[:].rearrange("p c s -> p (c s)"),
                    in_=ppd[:, NF:2048], func=AF.Copy)
                m1 = lop.tile([128, 2, NF], FP16, tag="m1")
                nc.vector.tensor_tensor(
                    out=m1[:, 0, :], in0=ppd[:, 0:NF],
                    in1=call[:, 1, :], op=ALU.max)
                nc.vector.tensor_tensor(
                    out=m1[:, 1, :], in0=call[:, 0, :],
                    in1=call[:, 2, :], op=ALU.max)
                m2 = lop.tile([128, NF], FP16, tag="m2")
                nc.vector.tensor_tensor(
                    out=m2[:], in0=m1[:, 0, :], in1=m1[:, 1, :], op=ALU.max)
                m3 = lop.tile([128, NF3], FP16, tag="m3")
                nc.vector.tensor_tensor(
                    out=m3[:], in0=m2[:, 0:NF3], in1=m2[:, NF3:NF],
                    op=ALU.max)
                # ---- top-8 slots
                top8 = lop.tile([128, K], FP16, tag="top8")
                nc.vector.max(out=top8[:], in_=m3[:])
                nc.vector.max_index(out=idx2[:, m, :], in_max=top8[:],
                                    in_values=m3[:])
                if DEBUG_B and t == 0:
                    nc.sync.dma_start(out=dbg_m3[:], in_=m3[:])

            # ---- per-tile 1024-idx gathers (list elem i = k*128 + p ->
            # wrapped [i%16, i//16]: idxs16[r, k*8+a] = idx2[16a+r, m, k])
            gth = bp.tile([128, 16, 64], FP, tag="gth")
            idxs16 = bp.tile([128, 2, 64], I16, tag="idxs16")
            for m in range(2):
                scr = dramp.tile([128, K], U16, tag=f"scr{m}")
                nc.sync.dma_start(out=scr[:], in_=idx2[:, m, :])
                nc.sync.dma_start(
                    out=idxs16[0:16, m, :].rearrange("r (k a) -> r k a", k=K),
                    in_=scr[:].rearrange("(a r) k -> r k a", r=16)
                        .bitcast(I16))
            scr2 = dramp.tile([16, 128], U16, tag="scr2")
            nc.sync.dma_start(
                out=scr2[:],
                in_=idxs16[0:16, :, :].rearrange("r m c -> r (m c)")
                    .bitcast(U16))
            nc.sync.dma_start(
                out=idxs16[:].rearrange("p m c -> p (m c)"),
                in_=scr2[:].rearrange("r c -> (r c)").unsqueeze(0)
                    .to_broadcast([8, 16 * 128]).bitcast(I16))
            for m in range(2):
                nc.gpsimd.dma_gather(
                    gth[:, m * 8:(m + 1) * 8, :], ptabs[b][:],
                    idxs16[:, m, :], 1024, 1024, 64)
            if DEBUG_B and b == 0:
                nc.sync.dma_start(
                    out=dbg_idx[:], in_=idx2[:, 0, :])
                nc.sync.dma_start(
                    out=dbg_gth[:],
                    in_=gth[:, 0:8, :].rearrange("p g v -> p (g v)"))

            for m in range(2):
                t = b * 2 + m
                gt = gth[:, m * 8:(m + 1) * 8, :].rearrange(
                    "p k (e c) -> p k e c", c=8)
                # ---- exact fp32 pd for the 64 candidates (Pool chains)
                p0 = lop.tile([128, K, 8], FP, tag="p0")
                nc.gpsimd.tensor_scalar(out=p0[:], in0=gt[:, :, :, 0],
                                        scalar1=rd_sb[:, t, 0:1],
                                        scalar2=None, op0=ALU.mult)
                p1 = lop.tile([128, K, 8], FP, tag="p1")
                nc.gpsimd.tensor_scalar(out=p1[:], in0=gt[:, :, :, 1],
                                        scalar1=rd_sb[:, t, 1:2],
                                        scalar2=None, op0=ALU.mult)
                nc.gpsimd.tensor_add(out=p0[:], in0=p0[:], in1=p1[:])
                nc.gpsimd.tensor_scalar(out=p1[:], in0=gt[:, :, :, 2],
                                        scalar1=rd_sb[:, t, 2:3],
                                        scalar2=None, op0=ALU.mult)
                nc.gpsimd.tensor_add(out=p0[:], in0=p0[:], in1=p1[:])
                pdc = lop.tile([128, K, 8], FP, tag="pdc")
                nc.gpsimd.tensor_sub(out=pdc[:], in0=p0[:],
                                     in1=gt[:, :, :, 3])
                # ---- threshold at 8th largest of the 64
                t8 = lop.tile([128, K], FP, tag="t8")
                nc.vector.max(out=t8[:],
                              in_=pdc[:].rearrange("p k e -> p (k e)"))
                r = lop.tile([128, K, 8], FP, tag="r")
                nc.scalar.activation(out=r[:], in_=pdc[:], func=AF.Relu,
                                     bias=t8[:, 7:8], scale=-1.0)
                # ---- masked max of y over the selected 8
                rb = lop.tile([128, K * 8], FP, tag="rb")
                nc.gpsimd.tensor_scalar(
                    out=rb[:], in0=r[:].rearrange("p k e -> p (k e)"),
                    scalar1=-BIG, scalar2=None, op0=ALU.mult)
                ys = lop.tile([128, OUT_F, K * 8], FP, tag="ys")
                nc.gpsimd.tensor_add(
                    out=ys[:],
                    in0=rb[:].unsqueeze(1).to_broadcast([128, OUT_F, K * 8]),
                    in1=gt[:, :, :, 4:7].rearrange("p k e v -> p v (k e)"))
                nc.vector.tensor_reduce(
                    out=final_raw[:, t, :], in_=ys[:],
                    axis=mybir.AxisListType.X, op=ALU.max)
                if DEBUG_B and t == 0:
                    nc.sync.dma_start(
                        out=dbg_pdc[:],
                        in_=pdc[:].rearrange("p k e -> p (k e)"))

        if DEBUG_B:
            nc.sync.dma_start(
                out=dbg_fr[:],
                in_=final_raw[:].rearrange("p t o -> p (t o)"))

        # ---- epilogue: += u, leaky, store
        nc.vector.scalar_tensor_tensor(
            out=final_raw[:], in0=final_raw[:], scalar=0.0,
            in1=rd_sb[:, :, 3:6], op0=ALU.bypass, op1=ALU.add)
        nc.vector.scalar_tensor_tensor(
            out=final_raw[:], in0=final_raw[:], scalar=0.2,
            in1=final_raw[:], op0=ALU.mult, op1=ALU.max)
        nc.sync.dma_start(
            out=outc[:].rearrange("b (m p) o -> p (b m) o", p=128),
            in_=final_raw[:])
    return nc


# --------------------------------------------------------------------------
# Host orchestration
# --------------------------------------------------------------------------
_CACHE = {}
LAST_RESULTS = {}


def _programs():
    if "a" not in _CACHE:
        nca = build_stage_a()
        nca.compile()
        ncb = build_stage_b()
        ncb.compile()
        _CACHE["a"] = nca
        _CACHE["b"] = ncb
    return _CACHE["a"], _CACHE["b"]


def _weight_folds(inputs):
    c1w = np.asarray(inputs["c1w"], np.float32)
    c1b = np.asarray(inputs["c1b"], np.float32)
    c2w = np.asarray(inputs["c2w"], np.float32)
    c2b = np.asarray(inputs["c2b"], np.float32)
    M1 = c1w[:, :3].T @ c2w.T                      # (3, 3)
    M2 = c1w[:, 3:].T @ c2w.T                      # (3, 3)
    zc = (c1b @ c2w.T + c2b).reshape(3)
    Wl = (np.asarray(inputs["Wl1"], np.float32)
          @ np.asarray(inputs["Wl2"], np.float32))  # (128, 3)
    wbig = np.zeros((IN_F, 16), np.float32)
    wbig[:, 0:3] = Wl
    wbig[:, 4:7] = Wl @ M1
    wbig[:, 8:11] = Wl @ (M2 - M1)
    abig = np.zeros((OUT_F, 16), np.float32)
    abig[:, 0:3] = np.eye(3, dtype=np.float32)
    abig[:, 4:7] = M1
    abig[:, 8:11] = M2 - M1
    bsel = np.zeros((OUT_F, 16), np.float32)
    bsel[:, 8:11] = np.eye(3, dtype=np.float32)
    biasd = np.asarray(inputs["bias"], np.float32).reshape(DEG, OUT_F)
    biasu = np.ascontiguousarray((biasd + zc.reshape(1, 3)).T)  # (3, 64)
    return wbig, abig, bsel, biasu


def _stage_a_inmaps(inputs):
    trees = [np.asarray(inputs[f"t{i}"], np.float32) for i in range(6)]
    wrs = [np.asarray(inputs[f"Wr{i}"], np.float32) for i in range(6)]
    wbf = np.asarray(inputs["W_branch"], np.float32).astype(np.float16)
    wbig, abig, bsel, biasu = _weight_folds(inputs)
    t5 = trees[5]
    in_maps = []
    for c in range(NCORES):
        m = {}
        nodes = [NLOC * c + j for j in range(NLOC)]
        for i in range(6):
            f = FEATS[i]
            nch = NCH[i]
            rows = [n * SIZES[i] // NODE for n in nodes]
            sl = trees[i][:, rows, :].transpose(2, 1, 0).reshape(f, NLOC * B)
            slp = np.zeros((nch * 128, NLOC * B), np.float32)
            slp[:f] = sl
            m[f"tlT{i}"] = np.ascontiguousarray(
                slp.reshape(nch, 128, NLOC * B).transpose(1, 0, 2)
                .reshape(128, nch * NLOC * B))
            wp = np.zeros((nch * 128, OUT_F), np.float32)
            wp[:f] = wrs[i]
            m[f"wr{i}"] = np.ascontiguousarray(
                wp.reshape(nch, 128, OUT_F).transpose(1, 0, 2)
                .reshape(128, nch * OUT_F))
        m["wb"] = np.ascontiguousarray(wbf[nodes])
        m["t5h"] = np.ascontiguousarray(
            t5[:, nodes, :].transpose(2, 1, 0)).astype(np.float16)
        m["wbig"] = wbig.astype(np.float16)
        m["abig"] = abig.astype(np.float16)
        m["bsel"] = bsel.astype(np.float16)
        m["biasu"] = biasu.astype(np.float16)
        in_maps.append(m)
    return in_maps


def _stage_b_inmaps(inputs, xouts, xout16s):
    # xouts: per-core (16, 4096) fp32; xout16s: per-core (8, 4096) fp16
    # cols (nl, b, d); global j = c*256 + nl*64 + d
    xs = np.stack([np.asarray(x).reshape(16, NLOC, B, DEG) for x in xouts])
    hs = np.stack([np.asarray(x).reshape(8, NLOC, B, DEG) for x in xout16s])
    allp = xs.transpose(1, 3, 0, 2, 4).reshape(16, B, N)     # fp32
    allh = hs.transpose(1, 3, 0, 2, 4).reshape(8, B, N)      # fp16
    # vall rows: xh, xl, xh, xxh2, xxl2, -1
    vall = np.empty((12, B, N), np.float16)
    vall[0:3] = allh[0:3]
    vall[3:6] = allh[4:7]
    vall[6:9] = allh[0:3]
    vall[9] = allh[3]
    vall[10] = allh[7]
    vall[11] = -1.0
    vall = np.ascontiguousarray(vall.reshape(12, B * N))
    ptabs = {}
    for b in range(B):
        pt = np.zeros((N, 8), np.float32)
        pt[:, 0:7] = allp[0:7, b].T       # x(3), xx2, y(3)
        # ptab8[s, e=(u*4+q), :] = pt[s + 256u + 512q]
        p8 = pt.reshape(2, 4, 256, 8).transpose(2, 0, 1, 3)  # (s, u, q, 8)
        ptabs[f"ptab{b}"] = np.ascontiguousarray(p8.reshape(256, 64))
    in_maps = []
    for c in range(NCORES):
        own = xs[c].transpose(0, 2, 1, 3).reshape(16, B, RLOC)
        ownh = hs[c].transpose(0, 2, 1, 3).reshape(8, NP_CORE)
        u = np.empty((12, NP_CORE), np.float16)
        u[0:3] = ownh[0:3]
        u[3:6] = ownh[0:3]
        u[6:9] = ownh[4:7]
        u[9:11] = -1.0
        u[11] = ownh[3]
        rd = np.zeros((128, 32, 8), np.float32)
        o2 = own.reshape(16, B, 2, 128)
        rd[:, :, 0:3] = o2[0:3].transpose(3, 1, 2, 0).reshape(128, 32, 3)
        rd[:, :, 3:6] = o2[8:11].transpose(3, 1, 2, 0).reshape(128, 32, 3)
        m = {"vall": vall, "uvl": np.ascontiguousarray(u),
             "rowdat": np.ascontiguousarray(rd.reshape(128, 32 * 8))}
        m.update(ptabs)
        in_maps.append(m)
    return in_maps


def kernel(**inputs):
    nca, ncb = _programs()
    core_ids = list(range(NCORES))

    ra = run_bass_kernel_spmd(nca, _stage_a_inmaps(inputs), core_ids)
    LAST_RESULTS["a"] = ra
    xouts = [np.asarray(ra.results[c]["xout"]) for c in range(NCORES)]
    xout16s = [np.asarray(ra.results[c]["xout16"]) for c in range(NCORES)]

    rb = run_bass_kernel_spmd(ncb, _stage_b_inmaps(inputs, xouts, xout16s),
                              core_ids)
    LAST_RESULTS["b"] = rb
    out = np.empty((B, N, OUT_F), np.float32)
    for c in range(NCORES):
        out[:, c * RLOC:(c + 1) * RLOC, :] = rb.results[c]["outc"]
    return out
